# revision 45
# baseline (speedup 1.0000x reference)
"""Trainium2 Bass kernel for nn_EquivariantScalar_viaTP.

Reference computation (after dead-code elimination — the gate / l=1 / l=2
paths never reach the output):

    s      = node_vec[:, :128]                                  # [N, 128]
    attr   = node_embedding                                     # [N, 32]
    s_mid  = einsum('nu,nv,uvw->nw', s, attr, W1s) / 64 + b1s   # [N, 128]
    s_act  = silu(s_mid)
    h      = einsum('nu,nv,uvw->nw', s_act, attr, W2) / 64 + b2 # [N, 32]
    h      = silu(h @ (W3/sqrt(32)) + b3)                       # [N, 32]
    out    = h @ (W4/sqrt(32)) + b4                             # [N, 1]

Sharding: node dim N=8192 across 8 cores (1024 nodes each).

Architecture (v3 — engine-balanced, no reduces / no transposes):

Stage 1 per node block: Z[(u,v), n] = s[n,u]*attr[n,v] built on the DVE
from host-staged replicated tiles (s rows 8x, attr rows 16x) in bf16 2x
mode; 32 PSUM-accumulating matmuls (k-tiles of 128) give s_mid^T [w, n];
ACT applies SiLU+b1s -> sactT bf16.

Stage 2: 8 matmuls with host-arranged lhsT W2x give t2T[(v,w2), n] in
PSUM, where partition p of tile t encodes v = 8*(t%4)+p%8 — exactly the
layout of the stage-1 attr tiles.  ACT drains t2T to SBUF bf16 and the
Pool engine applies the attr multiply (GPSIMD cannot touch PSUM); for
the last blocks the then-idle DVE multiplies straight out of PSUM.

Stage 3 folds the v-reduction INTO the PE: o3T[w3, n] accumulates 8
matmuls with lhsT W3x[p, w3] = (W3/sqrt(32))[w2(p), w3] — summing over
(v,w2) pairs reduces v and applies W3 in one pass.  b2 is folded into
the stage-3 bias (b3' = W3'^T b2 + b3), applied per-partition by the
ACT SiLU.  Stage 4: ones-row augmented matmul [W4'; b4] -> energy.

Scheduling: PE order is pinned instruction-by-instruction (sync=False
dep edges) into a software pipeline — cycle q runs stage-1 of block q,
stage-2 of q-1, stage-3 of q-2 and stage-4 of q-2 — sized so no engine
ever waits: blocks are [192, 320, 320, 192] nodes (small first/last blocks
shorten the DMA-latency head and the serial drain chain).  A sparse Pool->PE keep-alive chain
ramps the PE p-state to full clock before the first real matmul.
"""

import os

import numpy as np
import ml_dtypes

import concourse.bass as bass
import concourse.bacc as bacc
import concourse.mybir as mybir
from concourse.tile import TileContext, add_dep_helper
from concourse import bass_utils

N = 8192
P = 128          # partitions / MUL0
A = 32           # attr channels
NCORES = 8
NPC = N // NCORES          # 1024 nodes per core
NBLK = 4
BLKS = [192, 320, 320, 192]
OFFS = [0, 192, 512, 832]
KT = 32                    # k-tiles in stage-1 contraction (4096 / 128)
GS = 8                     # distinct s-side tiles per block
GA = 4                     # distinct attr-side tiles per block
NT2 = 8                    # stage-2/3 (v,w2) tiles
BLKM = max(BLKS)

F32 = mybir.dt.float32
BF16 = mybir.dt.bfloat16
BF = ml_dtypes.bfloat16

# sbc piece q: [ GA arep tiles | GS s tiles ] x [128, BLKS[q]]
PIECES = [(GS + GA) * b for b in BLKS]
PBASE = [0]
for b in BLKS:
    PBASE.append(PBASE[-1] + (GS + GA) * b)
SBC_COLS = PBASE[-1]

# mega_c (bf16): w2x [128, 8*128] | w3x [128, 8*32] | w4aug [33 rows, 1]
OFF_W2X = 0
OFF_W3X = OFF_W2X + NT2 * P
OFF_W4A = OFF_W3X + NT2 * A
FC = OFF_W4A + 1
# mega_b (f32): b1col [128,1] | b3fold [32 rows, 1]
OFF_B1 = 0
OFF_B3 = 1
FB = 2
FW = KT * P

_CACHE = {}
LAST_RESULT = None         # test harness reads exec_time_ns from here


def _build():
    nc = bacc.Bacc(trn_type="TRN2", target_bir_lowering=False, debug=False)

    mega_b_d = nc.dram_tensor("mega_b", [P, FB], F32, kind="ExternalInput")
    mega_w1_d = nc.dram_tensor("mega_w1", [P, FW], BF16, kind="ExternalInput")
    mega_c_d = nc.dram_tensor("mega_c", [P, FC], BF16, kind="ExternalInput")
    sbc_d = nc.dram_tensor("sbc", [P, SBC_COLS], BF16, kind="ExternalInput")
    out_d = nc.dram_tensor("out", [1, NPC], F32, kind="ExternalOutput")

    Alu = mybir.AluOpType
    Act = mybir.ActivationFunctionType

    with TileContext(nc) as tc:
        with (
            tc.tile_pool(name="const", bufs=1) as cp,
            tc.tile_pool(name="sbcp", bufs=4) as sbcp,
            tc.tile_pool(name="zp", bufs=2) as zp,
            tc.tile_pool(name="q2p", bufs=16) as q2p,
            tc.tile_pool(name="ps1", bufs=2, space="PSUM") as ps1,
            tc.tile_pool(name="ps2", bufs=5, space="PSUM") as ps2,
            tc.tile_pool(name="pst", bufs=1, space="PSUM") as pst,
        ):
            # ---- input DMAs, in the order compute consumes them ----
            sps = []
            w1o = FW // 4
            mega_w1 = cp.tile([P, FW], BF16, tag="mega_w1")
            for q in range(NBLK):
                sp = sbcp.tile([P, PIECES[q]], BF16, tag="sbc", name=f"sp{q}")
                sps.append(sp)

            def sbc_dma(q, lo, hi):
                nc.sync.dma_start(sps[q][:, lo:hi],
                                  sbc_d.ap()[:, PBASE[q] + lo:PBASE[q] + hi])

            mega_b = cp.tile([P, FB], F32, tag="mega_b")
            mega_c = cp.tile([P, FC], BF16, tag="mega_c")
            CUT0 = (GA + 2) * BLKS[0]      # areps + s-tiles 0,1
            sbc_dma(0, 0, CUT0)
            sbc_dma(0, CUT0, PIECES[0])
            nc.scalar.dma_start(mega_b[:], mega_b_d.ap())
            hq = w1o // 4
            nc.sync.dma_start(mega_w1[:, :hq], mega_w1_d.ap()[:, :hq])
            nc.sync.dma_start(mega_w1[:, hq:w1o], mega_w1_d.ap()[:, hq:w1o])
            nc.sync.dma_start(mega_w1[:, w1o:2 * w1o],
                              mega_w1_d.ap()[:, w1o:2 * w1o])
            sbc_dma(1, 0, PIECES[1] // 2)
            sbc_dma(1, PIECES[1] // 2, PIECES[1])
            nc.sync.dma_start(mega_w1[:, 2 * w1o:3 * w1o],
                              mega_w1_d.ap()[:, 2 * w1o:3 * w1o])
            nc.sync.dma_start(mega_w1[:, 3 * w1o:], mega_w1_d.ap()[:, 3 * w1o:])
            nc.scalar.dma_start(mega_c[:], mega_c_d.ap())
            for q in range(2, NBLK):
                h = PIECES[q] // 2
                sbc_dma(q, 0, h)
                sbc_dma(q, h, PIECES[q])

            b1col = mega_b[:, OFF_B1:OFF_B1 + 1]
            b3col = mega_b[0:A, OFF_B3:OFF_B3 + 1]
            w4aug = mega_c[0:A + 1, OFF_W4A:OFF_W4A + 1]

            sact = cp.tile([P, NPC], BF16, tag="sact")     # s_act^T [u, n]
            outsb = cp.tile([1, NPC], F32, tag="outsb")
            h3augs = []
            for i in range(2):
                h3a = cp.tile([P, BLKM], BF16, tag=f"h3aug{i}")
                nc.gpsimd.memset(h3a[A:A + 1, :], 1.0)
                h3augs.append(h3a)

            # PE p-state keep-alive: the tensor-engine clock ramps with time
            # since its first instruction (gaps under ~1us don't reset it).
            # A sparse Pool->PE chain from t~0.3us brings the PE to full
            # speed right as the first real matmul issues.
            wk = cp.tile([P, P], BF16, tag="wk")
            nc.gpsimd.memset(wk[:], 0.0)
            wacc = ps1.tile([P, BLKM], F32, tag="acc", name="wacc")
            pe_prev = [None]

            def pemm(out_ap, lhsT, rhs, start, stop):
                h = nc.tensor.matmul(out_ap, lhsT, rhs, start=start, stop=stop)
                if pe_prev[0] is not None:
                    add_dep_helper(h.ins, pe_prev[0].ins, sync=False,
                                   reason="pe order")
                pe_prev[0] = h
                return h

            act_prev = [None]

            def achain(h):
                act_prev[0] = h
                return h

            for i in range(6):
                if i:
                    nc.gpsimd.tensor_tensor(wk[:, :64], wk[:, :64],
                                            wk[:, 64:128], Alu.mult)
                pemm(wacc[:, :16], wk[:], wk[:, :16], start=True, stop=True)

            q2Ts = {}       # (q, t2) -> sbuf tile
            zs = {}
            accs = {}

            def emit_z(q):
                b = BLKS[q]
                sp = sps[q]
                z = zp.tile([P, KT * b], BF16, tag="z", name=f"z{q}")
                zs[q] = z
                z_v = z[:].rearrange("p (tv tu n) -> p tv tu n", tu=GS, n=b)
                a_rep3 = (sp[:, 0:GA * b]
                          .rearrange("p (tv n) -> p tv n", n=b))
                soff = GA * b
                for t in range(4):
                    s_pair = (sp[:, soff + 2 * t * b:soff + (2 * t + 2) * b]
                              .rearrange("p (tu n) -> p tu n", n=b)
                              .unsqueeze(1).broadcast_to([P, GA, 2, b]))
                    a_rep = a_rep3.unsqueeze(2).broadcast_to([P, GA, 2, b])
                    nc.vector.tensor_tensor(
                        z_v[:, :, 2 * t:2 * t + 2, :], s_pair, a_rep,
                        Alu.mult)

            def s1_group(q, t, j):
                b = BLKS[q]
                if q not in accs:
                    accs[q] = ps1.tile([P, b], F32, tag="acc", name=f"acc{q}")
                acc = accs[q]
                z_v = zs[q][:].rearrange("p (tv tu n) -> p tv tu n",
                                         tu=GS, n=b)
                for tv in range(GA):
                    sigma = 8 * t + 4 * j + tv
                    pemm(acc[:], mega_w1[:, sigma * P:(sigma + 1) * P],
                         z_v[:, tv, 2 * t + j, :],
                         start=(sigma == 0), stop=(sigma == KT - 1))

            def silu1(q):
                lo = OFFS[q]
                achain(nc.scalar.activation(sact[:, lo:lo + BLKS[q]],
                                             accs[q][:], Act.Silu,
                                             bias=b1col))

            def s2_one(q, t2):
                b = BLKS[q]
                lo = OFFS[q]
                if q == NBLK - 1 and t2 in (5, 6):
                    # tail: borrow the dead stage-1 acc banks for 7-in-flight
                    t2t = ps1.tile([P, b], F32, tag="acc",
                                   name=f"t2_{q}_{t2}")[:]
                else:
                    t2t = ps2.tile([P, b], F32, tag="t2",
                                   name=f"t2_{q}_{t2}")[:]
                pemm(t2t, mega_c[:, OFF_W2X + t2 * P:OFF_W2X + (t2 + 1) * P],
                     sact[:, lo:lo + b], start=True, stop=True)
                q2t = q2p.tile([P, b], BF16, tag="q2", name=f"q2_{q}_{t2}")
                arep = sps[q][:, (t2 % GA) * b:(t2 % GA + 1) * b]
                if q == NBLK - 1 or (q == NBLK - 2 and t2 % 2 == 1):
                    # late blocks: DVE (which may read PSUM) is idle by now
                    nc.vector.tensor_tensor(q2t[:], t2t, arep, Alu.mult)
                else:
                    # GPSIMD cannot access PSUM: ACT drains to SBUF bf16,
                    # Pool does the attr multiply in SBUF
                    t2s = q2p.tile([P, b], BF16, tag="t2s",
                                   name=f"t2s_{q}_{t2}")
                    achain(nc.scalar.copy(t2s[:], t2t))
                    nc.gpsimd.tensor_tensor(q2t[:], t2s[:], arep, Alu.mult)
                q2Ts[(q, t2)] = q2t

            o3ts = {}

            def s3_part(q, lo, hi):
                b = BLKS[q]
                if lo == 0:
                    o3ts[q] = pst.tile([A, b], F32, tag="ptr", name=f"o3_{q}")
                for t2 in range(lo, hi):
                    pemm(o3ts[q][:],
                         mega_c[:, OFF_W3X + t2 * A:OFF_W3X + (t2 + 1) * A],
                         q2Ts[(q, t2)][:],
                         start=(t2 == 0), stop=(t2 == NT2 - 1))
                if hi == NT2:
                    achain(nc.scalar.activation(h3augs[q % 2][0:A, :b],
                                                o3ts[q][:], Act.Silu,
                                                bias=b3col))

            def s3_all(q):
                s3_part(q, 0, NT2)

            out4s = {}

            def s4_mm(q):
                b = BLKS[q]
                out4s[q] = pst.tile([1, b], F32, tag="ptr", name=f"o4_{q}")
                pemm(out4s[q][:], w4aug, h3augs[q % 2][0:A + 1, :b],
                     start=True, stop=True)

            def s4_copy_dma(q):
                b = BLKS[q]
                lo = OFFS[q]
                achain(nc.scalar.copy(outsb[:, lo:lo + b], out4s[q][:]))
                nc.sync.dma_start(out_d.ap()[:, lo:lo + b],
                                  outsb[:, lo:lo + b])

            def s4_out(q):
                s4_mm(q)
                s4_copy_dma(q)

            GQ = [(t, j) for t in range(4) for j in range(2)]

            # ---- software pipeline with explicit PE ordering ----
            # cycle 0
            emit_z(0)
            for t, j in GQ:
                s1_group(0, t, j)
            silu1(0)
            # cycle 1 (z- and DMA-paced): [g0, g1, s2(0) 0-4, g2, g3,
            #                              s2 5, g4, g5, s2 6, g6, g7, s2 7]
            emit_z(1)
            s1_group(1, *GQ[0])
            s1_group(1, *GQ[1])
            for t2 in range(5):
                s2_one(0, t2)
            s1_group(1, *GQ[2])
            s1_group(1, *GQ[3])
            s2_one(0, 5)
            s1_group(1, *GQ[4])
            s1_group(1, *GQ[5])
            s2_one(0, 6)
            s1_group(1, *GQ[6])
            s1_group(1, *GQ[7])
            s2_one(0, 7)
            silu1(1)
            # cycles 2..3: [g0, g1, s2(q-1) 0-4, s3(q-2), s2 5, g2, s2 6, g3,
            #               s2 7, g4, s4(q-2), g5..g7]
            for q in (2, 3):
                emit_z(q)
                s1_group(q, *GQ[0])
                s1_group(q, *GQ[1])
                for t2 in range(5):
                    s2_one(q - 1, t2)
                s3_all(q - 2)
                gi = 2
                for t2 in range(5, NT2):
                    s2_one(q - 1, t2)
                    s1_group(q, *GQ[gi]); gi += 1
                s4_mm(q - 2)
                while gi < 8:
                    s1_group(q, *GQ[gi]); gi += 1
                silu1(q)
                s4_copy_dma(q - 2)
            # tail: interleave s3(2) with the block-3 stage-2 chain
            s3_part(2, 0, 4)
            for t2 in range(5):
                s2_one(3, t2)
            s3_part(2, 4, NT2)
            for t2 in range(5, NT2):
                s2_one(3, t2)
            s4_out(2)
            s3_all(3)
            s4_out(3)

    nc.compile()
    return nc


def _get_nc():
    if "nc" not in _CACHE:
        _CACHE["nc"] = _build()
    return _CACHE["nc"]


def _prep_inputs(node_vec, node_embedding, W1s, b1s, W2, b2, W3, b3, W4, b4):
    f = np.float32
    inv = f(1.0 / 64.0)                      # 1/sqrt(128*32)
    rsq = f(1.0) / np.sqrt(f(A))
    s = np.ascontiguousarray(node_vec[:, :P]).astype(f)
    attr = np.asarray(node_embedding, f)
    pidx = np.arange(P)

    # stage-1 weights: k-tile sigma = 8*t + 4*j + tv -> (tu = 2*t + j, tv)
    # partition p: u = 16*tu + p//8, v = 8*tv + p%8
    w1 = (np.asarray(W1s, f) * inv).astype(BF)           # [128u, 32v, 128w]
    mega_w1 = np.zeros((P, FW), BF)
    sigma = 0
    for t in range(4):
        for j in range(2):
            for tv in range(GA):
                tu = 2 * t + j
                u_of = 16 * tu + pidx // GS
                v_of = GS * tv + pidx % GS
                mega_w1[:, sigma * P:(sigma + 1) * P] = w1[u_of, v_of, :]
                sigma += 1

    # stage-2/3 weights: tile t2, partition/col c: v = 8*(t2%4) + c%8,
    # w2 = 16*(t2//4) + c//8
    w2 = np.asarray(W2, f) * inv                          # [128u, 32v, 32w]
    w3p = np.asarray(W3, f) * rsq                         # [32, 32]
    mega_c = np.zeros((P, FC), BF)
    cidx = np.arange(P)
    for t2 in range(NT2):
        v_c = GS * (t2 % GA) + cidx % GS
        w_c = 16 * (t2 // GA) + cidx // GS
        mega_c[:, OFF_W2X + t2 * P:OFF_W2X + (t2 + 1) * P] = \
            w2[:, v_c, w_c].astype(BF)
        mega_c[:, OFF_W3X + t2 * A:OFF_W3X + (t2 + 1) * A] = \
            w3p[w_c, :].astype(BF)
    mega_c[0:A, OFF_W4A] = (np.asarray(W4, f) * rsq)[:, 0].astype(BF)
    mega_c[A, OFF_W4A] = BF(np.asarray(b4, f).reshape(-1)[0])

    mega_b = np.zeros((P, FB), f)
    mega_b[:, OFF_B1] = np.asarray(b1s, f)
    mega_b[0:A, OFF_B3] = w3p.T @ np.asarray(b2, f) + np.asarray(b3, f)

    su_rows = 16 * np.arange(GS)[:, None] + (pidx // GS)[None, :]  # [GS, P]
    av_rows = GS * np.arange(GA)[:, None] + (pidx % GS)[None, :]   # [GA, P]

    in_maps = []
    for core in range(NCORES):
        lo = core * NPC
        S = s[lo:lo + NPC].astype(BF)                     # [1024, 128]
        atb = attr[lo:lo + NPC].astype(BF)                # [1024, 32]

        sbc = np.empty((P, SBC_COLS), BF)
        for q in range(NBLK):
            b = BLKS[q]
            Sb = S[OFFS[q]:OFFS[q] + b]                   # [b, 128]
            Ab = atb[OFFS[q]:OFFS[q] + b]                 # [b, 32]
            base = PBASE[q]
            for g in range(GA):
                sbc[:, base + g * b: base + (g + 1) * b] = Ab.T[av_rows[g]]
            base = PBASE[q] + GA * b
            for g in range(GS):
                sbc[:, base + g * b: base + (g + 1) * b] = Sb.T[su_rows[g]]

        in_maps.append(dict(mega_b=mega_b, mega_w1=mega_w1,
                            mega_c=mega_c, sbc=sbc))
    return in_maps


def kernel(**inputs):
    global LAST_RESULT
    trace = bool(int(os.environ.get("KERNEL_TRACE", "0")))
    in_maps = _prep_inputs(
        inputs["node_vec"], inputs["node_embedding"],
        inputs["W1s"], inputs["b1s"], inputs["W2"], inputs["b2"],
        inputs["W3"], inputs["b3"], inputs["W4"], inputs["b4"],
    )
    nc = _get_nc()
    res = bass_utils.run_bass_kernel_spmd(
        nc, in_maps, core_ids=list(range(NCORES)), trace=trace)
    LAST_RESULT = res
    outs = [np.asarray(res.results[i]["out"]) for i in range(NCORES)]
    energy = np.concatenate([o.reshape(NPC) for o in outs]).reshape(N, 1)
    return energy.astype(np.float32)
